# revision 26
# baseline (speedup 1.0000x reference)
"""Trainium2 Bass kernel for nn_MultiHeadAttention_39659728011763.

Shards the B*H=32 (batch, head-block) attention slices across 8 NeuronCores
(4 slices/core). The reference's raw .view head split makes slice r=16b+s use
token block [128s,128(s+1)) of batch b with ALL 1024 channels, so each core
only needs its 512 tokens of x plus the (replicated) projection weights.

Per-core pipeline (SPMD across 8 cores):
  0) x block [512,1024]: scale_norm (free-dim reduce) -> xn (f32r),
     PE-transpose -> xnT
  1) projections:
     q,k in transposed layout qT=[i,u] (lhsT=wT chunk, rhs=xnT); scale_norm
     over i via ACT-Square + PE-ones partition reduction; factor broadcast by
     rank-1 matmul; fused normalize+head-scatter into the packed QKP layout
     [64 x (u*16+h)] per (matrix, js-parity) partition group, all in f32r.
     v in natural layout [u,i]; scale_norm over free dim -> bf16; DMA
     head-reorg into V3P [(u%8,h), (js,sigma,c)].
  2) per slice js, A and B interleaved through a 2-slot [128,2048] PSUM
     ring (PSUM is fully budgeted: 8 banks = ring 2x4):
     A) S rows [128,2048] f32r -> PSUM; ACT exp(scale=1/tau) with accum_out
        = softmax denominator for free; DVE reciprocal + normalize;
        DMA attn rows (fp32) to HBM.
     B) S^T recompute [sigma-chunk,2048] f32r -> ACT exp -> bf16 expST.
     C) PV: out3T[c,t'1] += V3P_sigma.T @ expST_sigma (bf16, PSUM accum),
        written to HBM transposed; the tiny 1/denominator scale and the
        [c,t'1]->[t'1,c] transpose happen during host assembly (<1% of
        output bytes; the attn normalize, 99% of bytes, stays on-device).

Softmax skips max-subtraction: q,k rows are scale-normed to ||.||=32, so
|S|/tau <= 16 and exp stays comfortably inside fp32 range.
"""

import os

os.environ.setdefault("JAX_PLATFORMS", "cpu")

import numpy as np

import bass_rust as _bass_rust
import concourse.bass as bass
import concourse.mybir as mybir
import concourse.tile as _tile_mod
from concourse.bass_utils import run_bass_kernel_spmd
from concourse.masks import make_identity
from concourse.tile import TileContext
from concourse.vector_clock import ScopedClock

F32 = mybir.dt.float32
F32R = mybir.dt.float32r
BF16 = mybir.dt.bfloat16
AF = mybir.ActivationFunctionType

B, T, D = 2, 2048, 1024
H, DK = 16, 64
NCORES = 8
TOK = 512          # tokens per core
NS = 4             # slices (js) per core
TP = 2048          # t' extent per slice
SCALE = float(np.sqrt(D))            # 32.0
TAU_INV = 1.0 / float(np.sqrt(DK))   # 0.125


# ---------------------------------------------------------------------------
# Workarounds: the walrus build in this container accepts only ONE sync wait
# per instruction, while Tile's semaphore assignment attaches several.
# ---------------------------------------------------------------------------

def _patched_drain_and_barrier(self, tick_clock, wait_clock):
    nc = self.nc
    nop1 = nc.sync.nop(nofuse=True)
    wait_clock.add_sem_waits(nop1.ins, ScopedClock({None: tick_clock.global_clock}))
    si = nop1.ins.sync_info
    if si is not None and si.on_wait and len(si.on_wait) > 1:
        waits = list(si.on_wait)
        SI = type(si)
        nop1.ins.sync_info = SI(on_wait=[waits[0]], on_update=list(si.on_update))
        for w in waits[1:]:
            nc.sync.nop(nofuse=True).ins.sync_info = SI(on_wait=[w], on_update=[])
    nc.sync.drain()
    nc.all_engine_barrier()
    popped = nc._tile_sem_poison_stack.pop()
    assert popped is self._sem_poison
    nc.clear_and_free_semaphores(list(self.sems.allocated().values()))
    nc.all_engine_barrier()


_tile_mod.TileContext._drain_and_barrier = _patched_drain_and_barrier


def split_multi_waits(nc):
    """Hoist all-but-one sync wait of every instruction onto single-wait
    NoOps on the same engine, inserted just before it."""
    n_split = 0
    for f in nc.m.functions:
        for bb in f.blocks:
            new_list = []
            for ins in bb.instructions:
                si = getattr(ins, "sync_info", None)
                if si is not None and si.on_wait is not None and len(si.on_wait) > 1:
                    waits = list(si.on_wait)
                    SI = type(si)
                    for w in waits[:-1]:
                        nop = _bass_rust.InstNoOp(
                            name=nc.get_next_instruction_name(),
                            ins=[], outs=[], engine=ins.engine)
                        nop.sync_info = SI(on_wait=[w], on_update=[])
                        new_list.append(nop)
                    ins.sync_info = SI(on_wait=[waits[-1]], on_update=list(si.on_update))
                    n_split += 1
                new_list.append(ins)
            bb.instructions = new_list
    return n_split


# ---------------------------------------------------------------------------
# Kernel build
# ---------------------------------------------------------------------------

def build_nc():
    nc = bass.Bass()

    x_s = nc.dram_tensor("x_s", [TOK, D], F32, kind="ExternalInput")
    wqT = nc.dram_tensor("wqT", [D, D], F32R, kind="ExternalInput")
    wkT = nc.dram_tensor("wkT", [D, D], F32R, kind="ExternalInput")
    wvT = nc.dram_tensor("wvT", [D, D], F32R, kind="ExternalInput")
    bq = nc.dram_tensor("bq", [1, D], F32R, kind="ExternalInput")
    bk = nc.dram_tensor("bk", [1, D], F32R, kind="ExternalInput")
    bv = nc.dram_tensor("bv", [1, D], F32R, kind="ExternalInput")
    attn_s = nc.dram_tensor("attn_s", [NS, TP, TP], F32, kind="ExternalOutput")
    out3T_s = nc.dram_tensor("out3T_s", [NS, DK, TP], F32, kind="ExternalOutput")
    rdens_s = nc.dram_tensor("rdens_s", [128, NS * 16], F32, kind="ExternalOutput")

    with TileContext(nc, pool_alloc_mode="queue") as tc:
        with (
            tc.tile_pool(name="consts", bufs=1) as consts,
            tc.tile_pool(name="persist", bufs=1) as persist,
        ):
            # ---- constants ------------------------------------------------
            ident = consts.tile([128, 128], F32R, tag="ident")
            ones_col = consts.tile([128, 1], F32R, tag="ones_col")
            ones_row = consts.tile([1, 128], F32R, tag="ones_row")
            ones512 = consts.tile([1, 512], F32R, tag="ones512")
            bqt = consts.tile([1, D], F32R, tag="bqt")
            bkt = consts.tile([1, D], F32R, tag="bkt")
            bvt = consts.tile([1, D], F32R, tag="bvt")

            # ---- persistent tensors for phase 2 ---------------------------
            # QP/KP: packed head-transposed layout. Partition rows
            # [64*(js%2), +64) and column block (js//2)*2048 hold slice js as
            # [c, u_local*16 + h].
            QP = persist.tile([128, 4096], F32R, tag="QP")
            KP = persist.tile([128, 4096], F32R, tag="KP")
            V3P = persist.tile([128, NS, 16, DK], BF16, tag="V3P")
            rdens = persist.tile([128, NS * 16], F32, tag="rdens")

            # ================= phases 0 & 1 ================================
            with (
                tc.tile_pool(name="qraw", bufs=1) as qraw_pool,
                tc.tile_pool(name="work1", bufs=2) as work1,
                tc.tile_pool(name="xnp", bufs=1) as xnp,
                tc.tile_pool(name="xio", bufs=1) as xio,
                tc.tile_pool(name="wpool", bufs=2) as wpool,
                tc.tile_pool(name="small1", bufs=2) as small1,
                tc.tile_pool(name="ps1", bufs=1, space="PSUM") as ps1,
            ):
                identF = work1.tile([128, 128], F32, tag="identF", bufs=1)
                make_identity(nc, identF)
                nc.vector.tensor_copy(ident, identF)
                onesF = work1.tile([128, 512], F32, tag="onesF", bufs=1)
                nc.vector.memset(onesF, 1.0)
                nc.vector.tensor_copy(ones_col, onesF[:, 0:1])
                nc.vector.tensor_copy(ones_row, onesF[0:1, 0:128])
                nc.vector.tensor_copy(ones512, onesF[0:1, :])
                for dram_b, sb_b in ((bq, bqt), (bk, bkt), (bv, bvt)):
                    nc.sync.dma_start(out=sb_b, in_=dram_b[:, :])

                # ---- phase 0: x load, scale_norm, transpose ---------------
                xnT = xio.tile([128, 8, 512], F32R, tag="xnT")  # [d, (dchunk,u)]
                xn_tiles = []
                for m in range(4):
                    xt = work1.tile([128, D], F32, tag="xt")
                    nc.sync.dma_start(out=xt, in_=x_s[m * 128:(m + 1) * 128, :])
                    stats = small1.tile([128, 2, 6], F32, tag="xstats")
                    nc.vector.bn_stats(out=stats[:, 0, :], in_=xt[:, 0:512])
                    nc.vector.bn_stats(out=stats[:, 1, :], in_=xt[:, 512:1024])
                    mv = small1.tile([128, 2], F32, tag="xmv")
                    nc.vector.bn_aggr(out=mv, in_=stats)
                    msq = small1.tile([128, 1], F32, tag="xmsq")
                    nc.vector.tensor_mul(msq, mv[:, 0:1], mv[:, 0:1])
                    t2 = small1.tile([128, 1], F32, tag="xt2")
                    nc.vector.tensor_add(t2, msq, mv[:, 1:2])
                    nrm = small1.tile([128, 1], F32, tag="xn1")
                    nc.scalar.activation(out=nrm, in_=t2, func=AF.Sqrt, scale=float(D))
                    rn = small1.tile([128, 1], F32, tag="xrn")
                    nc.vector.reciprocal(rn, nrm)
                    rn32 = small1.tile([128, 1], F32, tag="xrn32")
                    nc.vector.tensor_scalar_mul(rn32, rn, SCALE)
                    xn = xnp.tile([128, D], F32R, tag=f"xn{m}")
                    nc.vector.tensor_scalar_mul(xn, xt, rn32)
                    xn_tiles.append(xn)
                for d in range(8):
                    pt = ps1.tile([128, 512], F32R, tag="xTp", bufs=3)
                    for m in range(4):
                        nc.tensor.transpose(
                            pt[:, m * 128:(m + 1) * 128],
                            xn_tiles[m][:, d * 128:(d + 1) * 128],
                            ident,
                        )
                    nc.vector.tensor_copy(xnT[:, d, :], pt)

                # ---- phase 1: q,k projection matmuls (transposed route) ---
                # All PE matmuls run back-to-back; factor chains and head
                # scatters are deferred / run on DVE+GPSIMD in their shadow.
                def proj_mms(wT_dram, b_sb):
                    wt = wpool.tile([128, 8, D], F32R, tag="w", name="wt")
                    for d in range(8):
                        nc.sync.dma_start(
                            out=wt[:, d, :], in_=wT_dram[d * 128:(d + 1) * 128, :])
                    ssp = ps1.tile([1, 512], F32, tag="ssp", bufs=1, name="ssp")
                    qraw = []
                    for beta in range(8):
                        pq = ps1.tile([128, 512], F32, tag="pq", bufs=3, name="pq")
                        for d in range(8):
                            nc.tensor.matmul(
                                pq, wt[:, d, beta * 128:(beta + 1) * 128],
                                xnT[:, d, :],
                                start=(d == 0), stop=False)
                        nc.tensor.matmul(
                            pq, b_sb[:, beta * 128:(beta + 1) * 128], ones512,
                            start=False, stop=True)
                        qr = qraw_pool.tile([128, 512], F32, tag=f"qraw{beta}",
                                            name="qr")
                        nc.vector.tensor_copy(qr, pq)
                        qraw.append(qr)
                        qsq = work1.tile([128, 512], F32R, tag="qsq", name="qsq")
                        nc.vector.tensor_mul(qsq, qr, qr)
                        nc.tensor.matmul(ssp, ones_col, qsq,
                                         start=(beta == 0), stop=(beta == 7),
                                         skip_group_check=True)
                    return ssp, qraw

                def factor_chain(ssp, fbc_tag):
                    # 32 / ||row||, broadcast to 128 partitions via DMA
                    ssb = small1.tile([1, 512], F32, tag="ssb", bufs=1, name="ssb")
                    nc.scalar.copy(ssb, ssp)
                    nrm = small1.tile([1, 512], F32, tag="qn1", bufs=1, name="nrm")
                    nc.scalar.activation(out=nrm, in_=ssb, func=AF.Sqrt)
                    rn = small1.tile([1, 512], F32, tag="qrn", bufs=1, name="rn")
                    nc.vector.reciprocal(rn, nrm)
                    rn32 = small1.tile([1, 512], F32R, tag="qrn32", bufs=1,
                                       name="rn32")
                    nc.vector.tensor_scalar_mul(rn32, rn, SCALE)
                    fbp = ps1.tile([128, 512], F32, tag="fbp", bufs=1, name="fbp")
                    nc.tensor.matmul(fbp, ones_row, rn32, start=True, stop=True)
                    fbc = work1.tile([128, 512], F32R, tag=fbc_tag, bufs=1,
                                     name="fbc")
                    nc.vector.tensor_copy(fbc, fbp)
                    return fbc

                def scatter(dstP, qraw, fbc, js, beta, par, engine):
                    view = dstP[
                        64 * (js % 2):64 * (js % 2) + 64,
                        (js // 2) * TP:(js // 2 + 1) * TP,
                    ].rearrange("p (u h) -> p u h", h=16)
                    h = 2 * beta + par
                    engine.tensor_mul(
                        view[:, :, h],
                        qraw[beta][par * 64:(par + 1) * 64,
                                   js * 128:(js + 1) * 128],
                        fbc[par * 64:(par + 1) * 64, js * 128:(js + 1) * 128],
                    )

                ssp_q, qraw_q = proj_mms(wqT, bqt)
                fbc_q = factor_chain(ssp_q, "fbcq")
                ssp_k, qraw_k = proj_mms(wkT, bkt)
                # q scatters: js0 on DVE (feeds slice-0 attention asap),
                # js1-3 on GPSIMD beta-major so k's PSUM->SBUF copies (which
                # reuse the qraw slots) unblock quickly
                for beta in range(8):
                    for par in range(2):
                        scatter(QP, qraw_q, fbc_q, 0, beta, par, nc.vector)
                for beta in range(8):
                    for par in range(2):
                        for js in range(1, NS):
                            scatter(QP, qraw_q, fbc_q, js, beta, par, nc.gpsimd)
                fbc_k = factor_chain(ssp_k, "fbck")
                for beta in range(8):
                    for par in range(2):
                        scatter(KP, qraw_k, fbc_k, 0, beta, par, nc.vector)
                for beta in range(8):
                    for par in range(2):
                        for js in range(1, NS):
                            scatter(KP, qraw_k, fbc_k, js, beta, par, nc.gpsimd)

                # ---- v projection (natural route) -> V3P ------------------
                wt = wpool.tile([128, 8, D], F32R, tag="w")
                for d in range(8):
                    nc.sync.dma_start(
                        out=wt[:, d, :], in_=wvT[d * 128:(d + 1) * 128, :])
                for m in range(4):
                    pv0 = ps1.tile([128, 512], F32, tag="pq", bufs=3)
                    pv1 = ps1.tile([128, 512], F32, tag="pq", bufs=3)
                    for nb2, pv in enumerate((pv0, pv1)):
                        for d in range(8):
                            nc.tensor.matmul(
                                pv, xnT[:, d, m * 128:(m + 1) * 128],
                                wt[:, d, nb2 * 512:(nb2 + 1) * 512],
                                start=(d == 0), stop=False)
                        nc.tensor.matmul(
                            pv, ones_row, bvt[:, nb2 * 512:(nb2 + 1) * 512],
                            start=False, stop=True)
                    stats = small1.tile([128, 2, 6], F32, tag="xstats")
                    nc.vector.bn_stats(out=stats[:, 0, :], in_=pv0)
                    nc.vector.bn_stats(out=stats[:, 1, :], in_=pv1)
                    mv = small1.tile([128, 2], F32, tag="xmv")
                    nc.vector.bn_aggr(out=mv, in_=stats)
                    msq = small1.tile([128, 1], F32, tag="xmsq")
                    nc.vector.tensor_mul(msq, mv[:, 0:1], mv[:, 0:1])
                    t2 = small1.tile([128, 1], F32, tag="xt2")
                    nc.vector.tensor_add(t2, msq, mv[:, 1:2])
                    nrm = small1.tile([128, 1], F32, tag="vn1")
                    nc.scalar.activation(out=nrm, in_=t2, func=AF.Sqrt, scale=float(D))
                    rn = small1.tile([128, 1], F32, tag="vrn")
                    nc.vector.reciprocal(rn, nrm)
                    rn32 = small1.tile([128, 1], F32, tag="vrn32")
                    nc.vector.tensor_scalar_mul(rn32, rn, SCALE)
                    vn = work1.tile([128, D], BF16, tag="vn", bufs=2)
                    nc.vector.tensor_scalar_mul(vn[:, 0:512], pv0, rn32)
                    nc.vector.tensor_scalar_mul(vn[:, 512:1024], pv1, rn32)
                    # head reorg: V3P[(du,h), m, sg, c] = vn[sg*8+du, h*64+c]
                    # spread across engine DMA queues to avoid serializing
                    dma_engines = (nc.sync, nc.gpsimd, nc.scalar, nc.sync)
                    for sg in range(16):
                        dma_engines[sg % 4].dma_start(
                            out=V3P[:, m, sg, :],
                            in_=vn[sg * 8:(sg + 1) * 8, :].rearrange(
                                "p (h c) -> p h c", c=DK),
                        )

            # ================= phase 2: attention ==========================
            with (
                tc.tile_pool(name="attn_io", bufs=1) as attn_io,
                tc.tile_pool(name="expst", bufs=1) as expst_pool,
                tc.tile_pool(name="work2", bufs=2) as work2,
                tc.tile_pool(name="small2", bufs=4) as small2,
                tc.tile_pool(name="ps2", bufs=1, space="PSUM") as ps2,
            ):
                for js in range(NS):
                    p0 = 64 * (js % 2)
                    c0 = (js // 2) * TP
                    q3 = QP[p0:p0 + 64, c0:c0 + TP]
                    k3 = KP[p0:p0 + 64, c0:c0 + TP]

                    # A/B interleaved: natural S rows (softmax+attn out) and
                    # transposed S (exp -> bf16 expST), sharing a 2-slot
                    # [128,2048] PSUM ring so PE stays ~2 tiles ahead of ACT.
                    expST = expst_pool.tile([128, 16, TP], BF16, tag="expST")
                    for t in range(16):
                        i1 = t
                        sn = ps2.tile([128, TP], F32, tag="big", bufs=2)
                        for nb in range(4):
                            nc.tensor.matmul(
                                sn[:, nb * 512:(nb + 1) * 512],
                                q3[:, i1 * 128:(i1 + 1) * 128],
                                k3[:, nb * 512:(nb + 1) * 512],
                                start=True, stop=True)
                        expt = attn_io.tile([128, TP], F32, tag="expt", bufs=3)
                        den = small2.tile([128, 1], F32, tag="den")
                        nc.scalar.activation(out=expt, in_=sn, func=AF.Exp,
                                             scale=TAU_INV, accum_out=den)
                        rd = rdens[:, js * 16 + i1:js * 16 + i1 + 1]
                        nc.vector.reciprocal(rd, den)
                        attn_t = attn_io.tile([128, TP], F32, tag="attn_t", bufs=3)
                        nc.vector.tensor_scalar_mul(attn_t, expt, rd)
                        nc.sync.dma_start(
                            out=attn_s[js, i1 * 128:(i1 + 1) * 128, :],
                            in_=attn_t)

                        sg = t
                        st = ps2.tile([128, TP], F32, tag="big", bufs=2)
                        for nb in range(4):
                            nc.tensor.matmul(
                                st[:, nb * 512:(nb + 1) * 512],
                                k3[:, sg * 128:(sg + 1) * 128],
                                q3[:, nb * 512:(nb + 1) * 512],
                                start=True, stop=True)
                        nc.scalar.activation(out=expST[:, sg, :], in_=st,
                                             func=AF.Exp, scale=TAU_INV)

                    # C) PV: out3T[c, t'1] accumulated over sigma chunks;
                    # the 1/denominator scale and the final transpose to
                    # [t'1, c] happen on the host (0.8% of output bytes).
                    o3 = ps2.tile([128, TP], F32, tag="big", bufs=2)
                    for nb in range(4):
                        for sg in range(16):
                            nc.tensor.matmul(
                                o3[0:64, nb * 512:(nb + 1) * 512],
                                V3P[:, js, sg, :],
                                expST[:, sg, nb * 512:(nb + 1) * 512],
                                start=(sg == 0), stop=(sg == 15))
                    o3s = work2.tile([64, TP], F32, tag="o3s", bufs=2)
                    nc.vector.tensor_copy(o3s, o3[0:64, :])
                    nc.sync.dma_start(out=out3T_s[js, :, :], in_=o3s)
                nc.sync.dma_start(out=rdens_s[:, :], in_=rdens)

    split_multi_waits(nc)
    return nc


_NC_CACHE = None
LAST_RESULTS = None


def _get_nc():
    global _NC_CACHE
    if _NC_CACHE is None:
        _NC_CACHE = build_nc()
    return _NC_CACHE


def kernel(x, scale, w_qs, b_qs, w_ks, b_ks, w_vs, b_vs, _trace=False):
    global LAST_RESULTS
    x = np.asarray(x, dtype=np.float32)
    wqT = np.ascontiguousarray(np.asarray(w_qs, np.float32).T)
    wkT = np.ascontiguousarray(np.asarray(w_ks, np.float32).T)
    wvT = np.ascontiguousarray(np.asarray(w_vs, np.float32).T)
    bq = np.ascontiguousarray(np.asarray(b_qs, np.float32).reshape(1, D))
    bk = np.ascontiguousarray(np.asarray(b_ks, np.float32).reshape(1, D))
    bv = np.ascontiguousarray(np.asarray(b_vs, np.float32).reshape(1, D))

    in_maps = []
    for c in range(NCORES):
        b_idx, j = divmod(c, NCORES // B)
        xs = np.ascontiguousarray(x[b_idx, j * TOK:(j + 1) * TOK, :])
        in_maps.append({
            "x_s": xs, "wqT": wqT, "wkT": wkT, "wvT": wvT,
            "bq": bq, "bk": bk, "bv": bv,
        })

    nc = _get_nc()
    res = run_bass_kernel_spmd(nc, in_maps, core_ids=list(range(NCORES)),
                               trace=_trace)
    LAST_RESULTS = res

    attn = np.concatenate([r["attn_s"] for r in res.results], axis=0)
    out3T = np.stack([r["out3T_s"] for r in res.results])   # [8, NS, DK, TP]
    rdens = np.stack([r["rdens_s"] for r in res.results])   # [8, 128, NS*16]
    # rden for slice (c, js) as a function of t'1 = i1*128 + p
    rd = rdens.reshape(NCORES, 128, NS, 16)                 # [c, p, js, i1]
    rd = np.transpose(rd, (0, 2, 3, 1)).reshape(NCORES, NS, TP)
    out3T = out3T * rd[:, :, None, :]                       # scale over t'1
    # out3T.reshape(32, DK, TP) IS swapaxes(out3, 1, 2) materialized, so the
    # reference's swapaxes(...).reshape(B, T, D) is a straight reshape here.
    out = np.ascontiguousarray(out3T, np.float32).reshape(B, T, D)
    return out, attn
